# revision 14
# baseline (speedup 1.0000x reference)
"""Biased cross-attention on 8 TRN2 NeuronCores.

Reference math (per batch b):
  qh/kh/vh = split_heads(x @ W.T + b)          H=8 heads, DH=64
  S = qh @ kh^T / sqrt(DH) + logits_bias       [H, Lq, Lk]
  P = softmax(S)  (key_padding_mask all-False)
  out = merge_heads(P @ vh) @ Wo.T + bo        [Lq, D]
  attn_head0 = P[0]                            [Lq, Lk]

Sharding: 8 shards = (b in 4) x (query-half in 2); each core handles one
(b, 512-query) slab with all heads/keys, so outputs concatenate with no
cross-core reduction.

Device layout: scores are computed transposed, S^T[lk, lq], so the P^T
tiles feed the P@V matmul directly (contraction dim lk on SBUF
partitions) with no on-chip transpose of P.  Softmax skips the max
subtraction (logits are O(+-6), exp is safe) and uses
exp(S/8 + bias) = exp(S/8) * exp(bias^T), with exp(bias^T) precomputed
once per core (bf16) and shared by all 8 heads.  Row sums ride along as
a 65th "ones" column appended to vh inside the P@V matmul.  P^T / expS /
vh are bf16 (DVE 2x mode, PE 1 cycle/row); S inputs stay float32r (full
throughput, near-fp32 precision).  attn_head0 is recomputed in [lq, lk]
orientation from qh/kh plus the untransposed bias slab — cheaper and
safer than transposing P^T on chip.
"""

import numpy as np

import concourse.bass as bass
import concourse.mybir as mybir
from concourse import bacc, tile
from concourse.bass_utils import run_bass_kernel_spmd

F32 = mybir.dt.float32
F32R = mybir.dt.float32r
BF16 = mybir.dt.bfloat16
EXP = mybir.ActivationFunctionType.Exp

B, LQ, LK, D = 4, 1024, 2048, 512
H, DH = 8, 64
LQS = LQ // 2          # queries per core
NKT = D // 128         # 4 contraction tiles over D
NJ = LK // 128         # 16 key tiles
NP = NJ // 2           # 8 key-tile pairs
NT = LQS // 128        # 4 query tiles (attn0)
SCALE = 1.0 / 8.0      # 1/sqrt(DH)

N_CORES = 8


def build_device_kernel(repeat=1):
    nc = bacc.Bacc("TRN2", target_bir_lowering=False, debug=False,
                   num_devices=N_CORES)

    qT = nc.dram_tensor("qT", [D, LQS], F32R, kind="ExternalInput").ap()
    kT = nc.dram_tensor("kT", [D, LK], F32R, kind="ExternalInput").ap()
    vT = nc.dram_tensor("vT", [D, LK], F32R, kind="ExternalInput").ap()
    biasT = nc.dram_tensor("biasT", [LK, LQS], F32, kind="ExternalInput").ap()
    bias0 = nc.dram_tensor("bias0", [LQS, LK], F32, kind="ExternalInput").ap()
    wqT = nc.dram_tensor("wqT", [D, D], F32R, kind="ExternalInput").ap()
    wkT = nc.dram_tensor("wkT", [D, D], F32R, kind="ExternalInput").ap()
    wvT = nc.dram_tensor("wvT", [D, D], F32R, kind="ExternalInput").ap()
    woT = nc.dram_tensor("woT", [D, D], F32R, kind="ExternalInput").ap()
    bq2 = nc.dram_tensor("bq2", [128, NKT], F32, kind="ExternalInput").ap()
    bk2 = nc.dram_tensor("bk2", [128, NKT], F32, kind="ExternalInput").ap()
    c02 = nc.dram_tensor("c02", [128, NKT], F32, kind="ExternalInput").ap()

    outT = nc.dram_tensor("outT", [D, LQS], F32, kind="ExternalOutput").ap()
    attn0 = nc.dram_tensor("attn0", [LQS, LK], F32, kind="ExternalOutput").ap()

    with tile.TileContext(nc) as tc:
        _body(nc, tc, qT, kT, vT, biasT, bias0, wqT, wkT, wvT, woT,
              bq2, bk2, c02, outT, attn0, repeat=repeat)
    nc.compile()
    return nc


def _body(nc, tc, qT, kT, vT, biasT, bias0, wqT, wkT, wvT, woT,
          bq2, bk2, c02, outT, attn0, repeat=1):
    ctxs = []

    def pool(**kw):
        p = tc.tile_pool(**kw)
        ctxs.append(p)
        return p.__enter__()

    # ---- pools (per-partition SBUF budget ~192KB) -----------------------
    chk = pool(name="chk", bufs=3)               # kT/vT 512-col chunks, 8KB ea
    wpool = pool(name="wpool", bufs=2)           # weights [128,4,512] 8KB ea
    smin = pool(name="smin", bufs=1)             # qT (8KB)
    osp = pool(name="osp", bufs=2)               # out staging [128,512] 2KB
    btmp = pool(name="btmp", bufs=2)             # biasT stream tiles 2KB
    keep = pool(name="keep", bufs=1)             # persistent
    wke = pool(name="wke", bufs=3)               # expS bf16 pairs 2KB
    wkp = pool(name="wkp", bufs=4)               # pt bf16 pairs 2KB
    wkt = pool(name="wkt", bufs=2)               # small vectors
    b0p = pool(name="b0p", bufs=2)               # attn0 bias tiles f32 8KB
    e0p = pool(name="e0p", bufs=1)               # attn0 E0 bf16 4KB
    a0p = pool(name="a0p", bufs=1)               # attn0 staging
    ps_mm = pool(name="ps_mm", bufs=2, space="PSUM")   # [128,2,512] 2 banks
    ps_o = pool(name="ps_o", bufs=2, space="PSUM")     # [65,512] 1 bank
    ps_rb = pool(name="ps_rb", bufs=1, space="PSUM")   # [64,512] 1 bank

    f32 = F32

    # ---- persistent tiles ----------------------------------------------
    khT_sb = keep.tile([128, NKT, LK], F32R, tag="khT")     # 32KB/part
    vh_sb = keep.tile([128, NJ, H, DH + 1], BF16, tag="vh")  # 16.3KB/part
    Et_sb = keep.tile([128, NJ, LQS], BF16, tag="Et")       # 16KB/part
    qhT_sb = keep.tile([128, NKT, LQS], F32R, tag="qhT")    # 8KB/part
    OT_sb = keep.tile([128, NKT, LQS], F32R, tag="OT")      # 8KB/part
    ones_sb = keep.tile([1, DH], F32R, tag="ones")
    onesb = keep.tile([128, NJ, H, 1], BF16, tag="onesb")
    bqv = keep.tile([128, NKT], f32, tag="bqv")
    bkv = keep.tile([128, NKT], f32, tag="bkv")
    c0v = keep.tile([128, NKT], f32, tag="c0v")

    onesf = keep.tile([1, DH], f32, tag="onesf")
    nc.vector.memset(onesf[:], 1.0)
    nc.vector.tensor_copy(out=ones_sb[:], in_=onesf[:])
    nc.vector.memset(onesb[:], 1.0)
    nc.sync.dma_start(out=bqv[:], in_=bq2)
    nc.sync.dma_start(out=bkv[:], in_=bk2)
    nc.sync.dma_start(out=c0v[:], in_=c02)
    # ones column of vh (col DH of every (j, h) slot)
    nc.vector.tensor_copy(out=vh_sb[:, :, :, DH:DH + 1], in_=onesb[:])

    r128 = lambda ap: ap.rearrange("(a p) n -> p a n", p=128)

    if repeat > 1:
        loop = tc.For_i(0, repeat, 1,
                        hint_engines=(mybir.EngineType.PE,
                                      mybir.EngineType.Activation,
                                      mybir.EngineType.DVE))
        loop.__enter__()
        ctxs.insert(0, loop)

    # ---- E^T = exp(biasT), bf16, shared across heads (early: fills ACT
    # while PE runs projections) ------------------------------------------
    for g in range(4):
        bt = btmp.tile([128, 4, LQS], f32, tag="bt")
        nc.gpsimd.dma_start(
            out=bt[:], in_=r128(biasT[g * 512:(g + 1) * 512, :]))
        nc.scalar.activation(out=Et_sb[:, 4 * g:4 * g + 4, :], in_=bt[:],
                             func=EXP)

    # ---- K projection: stream kT in 512-wide lk chunks ------------------
    # khT[d', lk] = sum_d Wk[d',d] k[lk,d] ; lhsT = wkT, rhs = kT
    wk_sb = wpool.tile([128, NKT, D], F32R, tag="w")
    nc.scalar.dma_start(out=wk_sb[:], in_=r128(wkT))
    for c in range(LK // 512):
        kc = chk.tile([128, NKT, 512], F32R, tag="chk")
        nc.sync.dma_start(out=kc[:], in_=r128(kT[:, c * 512:(c + 1) * 512]))
        for m in range(NKT):
            ps = ps_mm.tile([128, 2, 512], f32, tag="mm")
            for k in range(NKT):
                nc.tensor.matmul(
                    out=ps[:, 0, :],
                    lhsT=wk_sb[:, k, m * 128:(m + 1) * 128],
                    rhs=kc[:, k, :],
                    start=(k == 0), stop=(k == NKT - 1))
            nc.vector.tensor_scalar_add(
                khT_sb[:, m, c * 512:(c + 1) * 512], ps[:, 0, :],
                bkv[:, m:m + 1])

    # ---- Q projection ---------------------------------------------------
    qT_sb = smin.tile([128, NKT, LQS], F32R, tag="qT")
    nc.scalar.dma_start(out=qT_sb[:], in_=r128(qT))
    wq_sb = wpool.tile([128, NKT, D], F32R, tag="w")
    nc.gpsimd.dma_start(out=wq_sb[:], in_=r128(wqT))
    for m in range(NKT):
        ps = ps_mm.tile([128, 2, 512], f32, tag="mm")
        for k in range(NKT):
            nc.tensor.matmul(
                out=ps[:, 0, :],
                lhsT=wq_sb[:, k, m * 128:(m + 1) * 128],
                rhs=qT_sb[:, k, :],
                start=(k == 0), stop=(k == NKT - 1))
        nc.vector.tensor_scalar_add(qhT_sb[:, m, :], ps[:, 0, :],
                                    bqv[:, m:m + 1])

    # ---- attn_head0: recompute S0 in [lq, lk] orientation ---------------
    for t in range(NT):
        bt0 = b0p.tile([128, LK], f32, tag="b0")
        nc.gpsimd.dma_start(out=bt0[:], in_=bias0[t * 128:(t + 1) * 128, :])
        e0 = e0p.tile([128, LK], BF16, tag="e0")
        nc.scalar.activation(out=e0[:], in_=bt0[:], func=EXP)
        ex0 = a0p.tile([128, 4, 512], BF16, tag="ex0")
        for cc in range(2):
            ps = ps_mm.tile([128, 2, 512], f32, tag="mm")
            for c in range(2):
                ck = 2 * cc + c
                nc.tensor.matmul(
                    out=ps[:, c, :],
                    lhsT=qhT_sb[0:64, 0, t * 128:(t + 1) * 128],
                    rhs=khT_sb[0:64, 0, ck * 512:(ck + 1) * 512],
                    start=True, stop=True)
            nc.scalar.activation(
                out=ex0[:, 2 * cc:2 * cc + 2, :],
                in_=ps[:], func=EXP, scale=SCALE)
        p0 = a0p.tile([128, LK], BF16, tag="p0")
        nc.vector.tensor_mul(p0[:], ex0[:].rearrange("p a b -> p (a b)"),
                             e0[:])
        s0 = wkt.tile([128, 1], f32, tag="s0")
        nc.vector.tensor_reduce(out=s0[:], in_=p0[:],
                                axis=mybir.AxisListType.X,
                                op=mybir.AluOpType.add)
        r0 = wkt.tile([128, 1], f32, tag="r0")
        nc.vector.reciprocal(r0[:], s0[:])
        a0 = a0p.tile([128, LK], f32, tag="a0")
        nc.vector.tensor_scalar_mul(a0[:], p0[:], r0[:])
        nc.scalar.dma_start(out=attn0[t * 128:(t + 1) * 128, :], in_=a0[:])

    # ---- V projection:  vh[lk, d'] ; lhsT = vT chunk, rhs = wvT ---------
    wv_sb = wpool.tile([128, NKT, D], F32R, tag="w")
    nc.scalar.dma_start(out=wv_sb[:], in_=r128(wvT))
    for c in range(LK // 512):
        vc = chk.tile([128, NKT, 512], F32R, tag="chk")
        nc.sync.dma_start(out=vc[:], in_=r128(vT[:, c * 512:(c + 1) * 512]))
        for j in range(4 * c, 4 * c + 4):
            ps = ps_mm.tile([128, 2, 512], f32, tag="mm")
            jj = j - 4 * c
            for k in range(NKT):
                nc.tensor.matmul(
                    out=ps[:, 0, :],
                    lhsT=vc[:, k, jj * 128:(jj + 1) * 128],
                    rhs=wv_sb[:, k, :],
                    start=(k == 0), stop=(k == NKT - 1))
            nc.vector.tensor_copy(
                out=vh_sb[:, j, :, 0:DH],
                in_=ps[:, 0, :].rearrange("p (h d) -> p h d", h=H))

    # ---- attention: per head, stream key-tile pairs ---------------------
    wo_sb = wpool.tile([128, NKT, D], F32R, tag="w")
    nc.gpsimd.dma_start(out=wo_sb[:], in_=r128(woT))

    PIPE = 2  # PV lags S by this many pairs so PE never stalls on ACT/DVE
    for h in range(H):
        m, off = h // 2, 64 * (h % 2)
        o_ps = ps_o.tile([DH + 1, LQS], f32, tag="o")
        pts = [None] * NP

        def s_stage(p):
            ps = ps_mm.tile([128, 2, 512], f32, tag="mm")
            for c in range(2):
                j = 2 * p + c
                nc.tensor.matmul(
                    out=ps[:, c, :],
                    lhsT=khT_sb[off:off + 64, m, j * 128:(j + 1) * 128],
                    rhs=qhT_sb[off:off + 64, m, :],
                    start=True, stop=True)
            ex = wke.tile([128, 2, 512], BF16, tag="expS")
            nc.scalar.activation(out=ex[:], in_=ps[:], func=EXP, scale=SCALE)
            pt = wkp.tile([128, 2, 512], BF16, tag="pt")
            nc.vector.tensor_mul(pt[:], ex[:], Et_sb[:, 2 * p:2 * p + 2, :])
            pts[p] = pt

        def pv_stage(p):
            for c in range(2):
                j = 2 * p + c
                nc.tensor.matmul(
                    out=o_ps[:],
                    lhsT=vh_sb[:, j, h, :],
                    rhs=pts[p][:, c, :],
                    start=(j == 0), stop=(j == NJ - 1))

        for p in range(NP):
            s_stage(p)
            if p >= PIPE:
                pv_stage(p - PIPE)
        for p in range(NP - PIPE, NP):
            pv_stage(p)

        # normalize: OT rows of this head = o_ps[0:64] * (1/rowsum)
        rs = wkt.tile([1, LQS], F32R, tag="rs")
        with nc.allow_low_precision(reason="f32r rowsum reciprocal"):
            nc.vector.reciprocal(rs[:], o_ps[DH:DH + 1, :])
        rb = ps_rb.tile([DH, LQS], f32, tag="rb")
        nc.tensor.matmul(out=rb[:], lhsT=ones_sb[:], rhs=rs[:],
                         start=True, stop=True)
        rb_sb = wkt.tile([DH, LQS], f32, tag="rbs")
        nc.vector.tensor_copy(out=rb_sb[:], in_=rb[:])
        nc.vector.tensor_mul(OT_sb[off:off + 64, m, :], o_ps[0:DH, :],
                             rb_sb[:])

    # ---- output projection ---------------------------------------------
    for mm in range(NKT):
        ps = ps_mm.tile([128, 2, 512], f32, tag="mm")
        for k in range(NKT):
            nc.tensor.matmul(
                out=ps[:, 0, :],
                lhsT=wo_sb[:, k, mm * 128:(mm + 1) * 128],
                rhs=OT_sb[:, k, :],
                start=(k == 0), stop=(k == NKT - 1))
        ot = osp.tile([128, LQS], f32, tag="osb")
        nc.vector.tensor_scalar_add(ot[:], ps[:, 0, :], c0v[:, mm:mm + 1])
        nc.sync.dma_start(out=outT[mm * 128:(mm + 1) * 128, :], in_=ot[:])

    for p in reversed(ctxs):
        p.__exit__(None, None, None)


_NC = None


def _get_nc():
    global _NC
    if _NC is None:
        _NC = build_device_kernel()
    return _NC


def make_in_maps(q, k, v, logits_bias, Wq, bq, Wk, bk, Wv, bv, Wo, bo,
                 key_padding_mask):
    q = np.asarray(q, np.float32)
    k = np.asarray(k, np.float32)
    v = np.asarray(v, np.float32)
    logits_bias = np.asarray(logits_bias, np.float32)
    mask = np.asarray(key_padding_mask)
    if mask.any():
        logits_bias = logits_bias + np.where(mask, -1e30, 0.0)[:, None, :] \
            .astype(np.float32)

    qT = np.ascontiguousarray(q.transpose(0, 2, 1))
    kT = np.ascontiguousarray(k.transpose(0, 2, 1))
    vT = np.ascontiguousarray(v.transpose(0, 2, 1))
    biasT = np.ascontiguousarray(logits_bias.transpose(0, 2, 1))

    wqT = np.ascontiguousarray(np.asarray(Wq, np.float32).T)
    wkT = np.ascontiguousarray(np.asarray(Wk, np.float32).T)
    wvT = np.ascontiguousarray(np.asarray(Wv, np.float32).T)
    woT = np.ascontiguousarray(np.asarray(Wo, np.float32).T)
    c0 = (np.asarray(Wo, np.float32) @ np.asarray(bv, np.float32)
          + np.asarray(bo, np.float32))
    arr2 = lambda x: np.ascontiguousarray(
        np.asarray(x, np.float32).reshape(NKT, 128).T)
    bq2, bk2, c02 = arr2(bq), arr2(bk), arr2(c0)

    in_maps = []
    for cidx in range(N_CORES):
        b, half = divmod(cidx, 2)
        sl = slice(half * LQS, (half + 1) * LQS)
        in_maps.append({
            "qT": np.ascontiguousarray(qT[b][:, sl]),
            "kT": kT[b], "vT": vT[b],
            "biasT": np.ascontiguousarray(biasT[b][:, sl]),
            "bias0": np.ascontiguousarray(logits_bias[b][sl, :]),
            "wqT": wqT, "wkT": wkT, "wvT": wvT, "woT": woT,
            "bq2": bq2, "bk2": bk2, "c02": c02,
        })
    return in_maps


def assemble(results):
    out = np.empty((B, LQ, D), np.float32)
    attn_head0 = np.empty((B, LQ, LK), np.float32)
    for cidx in range(N_CORES):
        b, half = divmod(cidx, 2)
        sl = slice(half * LQS, (half + 1) * LQS)
        out[b, sl, :] = results[cidx]["outT"].T
        attn_head0[b, sl, :] = results[cidx]["attn0"]
    return out, attn_head0


def kernel(**inputs):
    nc = _get_nc()
    in_maps = make_in_maps(**inputs)
    res = run_bass_kernel_spmd(nc, in_maps, core_ids=list(range(N_CORES)))
    return assemble(res.results)
